# revision 7
# baseline (speedup 1.0000x reference)
"""Int32 3x3 conv2d (stride 1, pad 1) as bf16 matmuls on 8 TRN2 cores.

Problem: x[16,256,56,56] (*) w[256,256,3,3] + b[256] -> y[16,256,56,56],
all int32, values in [0,127).

Trick: values 0..126 are exactly representable in bf16, every product is
an integer < 2^14, and every accumulation stays < 2^24, so a bf16 matmul
with fp32 PSUM accumulation produces bit-exact integer results.

Layout: each image is zero-padded to 58x58. The 3x3 conv becomes 9
shifted [Cin,Cout]^T @ [Cin,pixels] matmuls accumulated in PSUM; pixel
tiles are 8 output rows x 56 cols = 448 columns (one PSUM bank), read
from the padded image through a strided access pattern so only valid
pixels are computed.

Sharding: data-parallel over batch, 2 images per core; weights replicated.
"""

import numpy as np
import ml_dtypes

B, C, H, W = 16, 256, 56, 56
HP, WP = H + 2, W + 2          # 58, 58 padded
IMG = HP * WP                  # 3364 flat padded image
N_CORES = 8
IMG_PER_CORE = B // N_CORES    # 2
ROWS_PER_CHUNK = 8
CHUNK = ROWS_PER_CHUNK * W     # 448 valid pixels, fits one PSUM bank
N_CHUNKS = H // ROWS_PER_CHUNK  # 7
N_WARM = 9                     # cold matmuls to flip the HAM clock gate

_BF16 = ml_dtypes.bfloat16


def _build_program():
    import concourse.bass as bass
    import concourse.mybir as mybir
    from concourse import bacc
    from concourse.tile import TileContext

    nc = bacc.Bacc("TRN2", target_bir_lowering=False, debug=False)

    x_h = nc.dram_tensor(
        "x", [2, 128, IMG_PER_CORE * IMG], mybir.dt.bfloat16,
        kind="ExternalInput",
    )
    w_h = nc.dram_tensor(
        "w", [128, 2 * 2 * 9 * 128], mybir.dt.bfloat16, kind="ExternalInput"
    )
    b_h = nc.dram_tensor("b", [128, 2], mybir.dt.float32, kind="ExternalInput")
    y_h = nc.dram_tensor(
        "y", [IMG_PER_CORE, 2, 128, H, W], mybir.dt.int32, kind="ExternalOutput"
    )

    with TileContext(nc) as tc:
        with (
            tc.tile_pool(name="const", bufs=1) as const_pool,
            tc.tile_pool(name="xin", bufs=1) as x_pool,
            tc.tile_pool(name="psum", bufs=5, space="PSUM") as psum_pool,
            tc.tile_pool(name="warm", bufs=1, space="PSUM") as warm_pool,
            tc.tile_pool(name="outs", bufs=3) as out_pool,
        ):
            # PE warm-up: ~3.4us of junk matmuls on a zeroed tile while the
            # input DMAs land, so the HAM clock gate is at 8/8 (2.4 GHz)
            # when the real matmuls start.
            wz = const_pool.tile([128, 128 + CHUNK], mybir.dt.bfloat16)
            nc.vector.memset(wz[:, :], 0.0)
            wps = warm_pool.tile([128, CHUNK], mybir.dt.float32)
            for i in range(N_WARM):
                nc.tensor.matmul(
                    wps[:, :], wz[:, 0:128], wz[:, 128:128 + CHUNK],
                    start=True, stop=True,
                )

            # Input DMAs: one w tile per (ci_chunk, co_chunk) and one x tile
            # per (ci_chunk, img), so each matmul gates on exactly the data
            # it reads. Issues are spread across engine sequencers (a DMA
            # trigger costs ~0.6us of sequencer time) with the first matmul
            # group's tensors (w00, x00) issued first and in parallel.
            w_sb = {}
            for ci in range(2):
                for co in range(2):
                    w_sb[ci, co] = const_pool.tile(
                        [128, 9 * 128], mybir.dt.bfloat16,
                        tag=f"w_{ci}_{co}", name=f"w_{ci}_{co}",
                    )

            def w_dma(eng, ci, co):
                s = (ci * 2 + co) * 9 * 128
                eng.dma_start(w_sb[ci, co][:, :], w_h.ap()[:, s:s + 9 * 128])

            x_t = {}
            for img in range(IMG_PER_CORE):
                for ci in range(2):
                    x_t[ci, img] = x_pool.tile(
                        [128, IMG], mybir.dt.bfloat16,
                        tag=f"x_{ci}_{img}", name=f"x_{ci}_{img}",
                    )

            def x_dma(eng, ci, img):
                eng.dma_start(
                    x_t[ci, img][:, :],
                    x_h.ap()[ci][:, img * IMG:(img + 1) * IMG],
                )

            b_sb = const_pool.tile([128, 2], mybir.dt.float32)

            w_dma(nc.gpsimd, 0, 0)
            x_dma(nc.sync, 0, 0)
            w_dma(nc.scalar, 1, 0)
            x_dma(nc.gpsimd, 1, 0)
            w_dma(nc.scalar, 0, 1)
            w_dma(nc.gpsimd, 1, 1)
            x_dma(nc.sync, 0, 1)
            x_dma(nc.gpsimd, 1, 1)
            nc.scalar.dma_start(b_sb[:, :], b_h.ap())

            x_sb = {
                k: t[:, :].rearrange("p (r c) -> p r c", c=WP)
                for k, t in x_t.items()
            }

            for img in range(IMG_PER_CORE):
                for co in range(2):
                    for pc in range(N_CHUNKS):
                        r0 = pc * ROWS_PER_CHUNK
                        ps = psum_pool.tile([128, CHUNK], mybir.dt.float32)
                        n_mm = 0
                        for ci in range(2):
                            for k in range(9):
                                kh, kw = divmod(k, 3)
                                lhsT = w_sb[ci, co][:, k * 128:(k + 1) * 128]
                                rhs = x_sb[ci, img][
                                    :, r0 + kh:r0 + kh + ROWS_PER_CHUNK,
                                    kw:kw + W,
                                ]
                                nc.tensor.matmul(
                                    ps[:, :], lhsT, rhs,
                                    start=(n_mm == 0), stop=(n_mm == 17),
                                )
                                n_mm += 1
                        ot = out_pool.tile([128, CHUNK], mybir.dt.int32)
                        nc.vector.tensor_scalar_add(
                            ot[:, :], ps[:, :], b_sb[:, co:co + 1]
                        )
                        dst = y_h.ap()[img, co].rearrange("p h w -> p (h w)")[
                            :, pc * CHUNK:(pc + 1) * CHUNK
                        ]
                        nc.sync.dma_start(dst, ot[:, :])

    nc.compile()
    return nc


_NC = None
LAST_RESULT = None  # BassKernelResults of the most recent run (for harnesses)


def kernel(x_int: np.ndarray, weight_int: np.ndarray, bias_int: np.ndarray):
    from concourse.bass_utils import run_bass_kernel_spmd

    global _NC, LAST_RESULT
    if _NC is None:
        _NC = _build_program()
    nc = _NC

    x_int = np.asarray(x_int)
    weight_int = np.asarray(weight_int)
    bias_int = np.asarray(bias_int)

    # x: pad to 58x58, cast to bf16, split channels into two 128-partition
    # chunks: per core [ci_chunk, 128, img, IMG].
    x_pad = np.zeros((B, C, HP, WP), dtype=_BF16)
    x_pad[:, :, 1:57, 1:57] = x_int.astype(_BF16)
    x_flat = x_pad.reshape(B, 2, 128, IMG)

    # w[co,ci,kh,kw] -> [ci_part, ci_chunk, co_chunk, k, co_part]
    w_t = (
        weight_int.astype(_BF16)
        .reshape(2, 128, 2, 128, 9)          # [co_c, co_p, ci_c, ci_p, k]
        .transpose(3, 2, 0, 4, 1)            # [ci_p, ci_c, co_c, k, co_p]
        .reshape(128, 2 * 2 * 9 * 128)
    )
    w_t = np.ascontiguousarray(w_t)
    b_t = np.ascontiguousarray(
        bias_int.astype(np.float32).reshape(2, 128).T
    )

    in_maps = []
    for c in range(N_CORES):
        xc = np.ascontiguousarray(
            x_flat[c * IMG_PER_CORE:(c + 1) * IMG_PER_CORE].transpose(1, 2, 0, 3)
        )  # [ci_chunk, 128, img, IMG]
        in_maps.append(
            {
                "x": xc.reshape(2, 128, IMG_PER_CORE * IMG),
                "w": w_t,
                "b": b_t,
            }
        )

    res = run_bass_kernel_spmd(nc, in_maps, core_ids=list(range(N_CORES)))
    LAST_RESULT = res

    y = np.empty((B, C, H, W), dtype=np.int32)
    for c in range(N_CORES):
        yc = res.results[c]["y"]  # [img, co_chunk, 128, H, W]
        for img in range(IMG_PER_CORE):
            y[c * IMG_PER_CORE + img] = yc[img].reshape(C, H, W)
    return y


# revision 8
# speedup vs baseline: 1.0541x; 1.0541x over previous
"""Int32 3x3 conv2d (stride 1, pad 1) as bf16 matmuls on 8 TRN2 cores.

Problem: x[16,256,56,56] (*) w[256,256,3,3] + b[256] -> y[16,256,56,56],
all int32, values in [0,127).

Trick: values 0..126 are exactly representable in bf16, every product is
an integer < 2^14, and every accumulation stays < 2^24, so a bf16 matmul
with fp32 PSUM accumulation produces bit-exact integer results.

Layout: each image is zero-padded to 58x58. The 3x3 conv becomes 9
shifted [Cin,Cout]^T @ [Cin,pixels] matmuls accumulated in PSUM; pixel
tiles are 8 output rows x 56 cols = 448 columns (one PSUM bank), read
from the padded image through a strided access pattern so only valid
pixels are computed.

Sharding: data-parallel over batch, 2 images per core; weights replicated.
"""

import numpy as np
import ml_dtypes

B, C, H, W = 16, 256, 56, 56
HP, WP = H + 2, W + 2          # 58, 58 padded
IMG = HP * WP                  # 3364 flat padded image
N_CORES = 8
IMG_PER_CORE = B // N_CORES    # 2
ROWS_PER_CHUNK = 8
CHUNK = ROWS_PER_CHUNK * W     # 448 valid pixels, fits one PSUM bank
N_CHUNKS = H // ROWS_PER_CHUNK  # 7
N_WARM = 9                     # cold matmuls to flip the HAM clock gate

_BF16 = ml_dtypes.bfloat16


def _build_program():
    import concourse.bass as bass
    import concourse.mybir as mybir
    from concourse import bacc
    from concourse.tile import TileContext

    nc = bacc.Bacc("TRN2", target_bir_lowering=False, debug=False)

    x_h = nc.dram_tensor(
        "x", [2, 128, IMG_PER_CORE * IMG], mybir.dt.bfloat16,
        kind="ExternalInput",
    )
    w_h = nc.dram_tensor(
        "w", [128, 2 * 2 * 9 * 128], mybir.dt.bfloat16, kind="ExternalInput"
    )
    b_h = nc.dram_tensor("b", [128, 2], mybir.dt.float32, kind="ExternalInput")
    y_h = nc.dram_tensor(
        "y", [IMG_PER_CORE, 2, 128, H, W], mybir.dt.int32, kind="ExternalOutput"
    )

    with TileContext(nc) as tc:
        with (
            tc.tile_pool(name="const", bufs=1) as const_pool,
            tc.tile_pool(name="xin", bufs=1) as x_pool,
            tc.tile_pool(name="psum", bufs=5, space="PSUM") as psum_pool,
            tc.tile_pool(name="warm", bufs=1, space="PSUM") as warm_pool,
            tc.tile_pool(name="outs", bufs=3) as out_pool,
        ):
            # PE warm-up: ~3.4us of junk matmuls on a zeroed tile while the
            # input DMAs land, so the HAM clock gate is at 8/8 (2.4 GHz)
            # when the real matmuls start.
            wz = const_pool.tile([128, 128 + CHUNK], mybir.dt.bfloat16)
            nc.vector.memset(wz[:, :], 0.0)
            wps = warm_pool.tile([128, CHUNK], mybir.dt.float32)
            for i in range(N_WARM):
                nc.tensor.matmul(
                    wps[:, :], wz[:, 0:128], wz[:, 128:128 + CHUNK],
                    start=True, stop=True,
                )

            # Input DMAs: one w tile per (ci_chunk, co_chunk) and one x tile
            # per (ci_chunk, img), so each matmul gates on exactly the data
            # it reads. Issues are spread across engine sequencers (a DMA
            # trigger costs ~0.6us of sequencer time) with the first matmul
            # group's tensors (w00, x00) issued first and in parallel.
            w_sb = {}
            for ci in range(2):
                for co in range(2):
                    w_sb[ci, co] = const_pool.tile(
                        [128, 9 * 128], mybir.dt.bfloat16,
                        tag=f"w_{ci}_{co}", name=f"w_{ci}_{co}",
                    )

            def w_dma(eng, ci, co):
                s = (ci * 2 + co) * 9 * 128
                eng.dma_start(w_sb[ci, co][:, :], w_h.ap()[:, s:s + 9 * 128])

            x_t = {}
            for img in range(IMG_PER_CORE):
                for ci in range(2):
                    x_t[ci, img] = x_pool.tile(
                        [128, IMG], mybir.dt.bfloat16,
                        tag=f"x_{ci}_{img}", name=f"x_{ci}_{img}",
                    )

            def x_dma(eng, ci, img):
                eng.dma_start(
                    x_t[ci, img][:, :],
                    x_h.ap()[ci][:, img * IMG:(img + 1) * IMG],
                )

            b_sb = const_pool.tile([128, 2], mybir.dt.float32)

            # Two issue streams in first-needed order: DMA queues are FIFO,
            # so earlier transfers drain at full bandwidth before later
            # ones start, instead of fair-sharing with not-yet-needed data.
            x_dma(nc.sync, 0, 0)
            w_dma(nc.scalar, 0, 0)
            x_dma(nc.sync, 1, 0)
            w_dma(nc.scalar, 1, 0)
            x_dma(nc.sync, 0, 1)
            w_dma(nc.scalar, 0, 1)
            x_dma(nc.sync, 1, 1)
            w_dma(nc.scalar, 1, 1)
            nc.scalar.dma_start(b_sb[:, :], b_h.ap())

            x_sb = {
                k: t[:, :].rearrange("p (r c) -> p r c", c=WP)
                for k, t in x_t.items()
            }

            for img in range(IMG_PER_CORE):
                for co in range(2):
                    for pc in range(N_CHUNKS):
                        r0 = pc * ROWS_PER_CHUNK
                        ps = psum_pool.tile([128, CHUNK], mybir.dt.float32)
                        n_mm = 0
                        for ci in range(2):
                            for k in range(9):
                                kh, kw = divmod(k, 3)
                                lhsT = w_sb[ci, co][:, k * 128:(k + 1) * 128]
                                rhs = x_sb[ci, img][
                                    :, r0 + kh:r0 + kh + ROWS_PER_CHUNK,
                                    kw:kw + W,
                                ]
                                nc.tensor.matmul(
                                    ps[:, :], lhsT, rhs,
                                    start=(n_mm == 0), stop=(n_mm == 17),
                                )
                                n_mm += 1
                        ot = out_pool.tile([128, CHUNK], mybir.dt.int32)
                        nc.vector.tensor_scalar_add(
                            ot[:, :], ps[:, :], b_sb[:, co:co + 1]
                        )
                        dst = y_h.ap()[img, co].rearrange("p h w -> p (h w)")[
                            :, pc * CHUNK:(pc + 1) * CHUNK
                        ]
                        nc.sync.dma_start(dst, ot[:, :])

    nc.compile()
    return nc


_NC = None
LAST_RESULT = None  # BassKernelResults of the most recent run (for harnesses)


def kernel(x_int: np.ndarray, weight_int: np.ndarray, bias_int: np.ndarray):
    from concourse.bass_utils import run_bass_kernel_spmd

    global _NC, LAST_RESULT
    if _NC is None:
        _NC = _build_program()
    nc = _NC

    x_int = np.asarray(x_int)
    weight_int = np.asarray(weight_int)
    bias_int = np.asarray(bias_int)

    # x: pad to 58x58, cast to bf16, split channels into two 128-partition
    # chunks: per core [ci_chunk, 128, img, IMG].
    x_pad = np.zeros((B, C, HP, WP), dtype=_BF16)
    x_pad[:, :, 1:57, 1:57] = x_int.astype(_BF16)
    x_flat = x_pad.reshape(B, 2, 128, IMG)

    # w[co,ci,kh,kw] -> [ci_part, ci_chunk, co_chunk, k, co_part]
    w_t = (
        weight_int.astype(_BF16)
        .reshape(2, 128, 2, 128, 9)          # [co_c, co_p, ci_c, ci_p, k]
        .transpose(3, 2, 0, 4, 1)            # [ci_p, ci_c, co_c, k, co_p]
        .reshape(128, 2 * 2 * 9 * 128)
    )
    w_t = np.ascontiguousarray(w_t)
    b_t = np.ascontiguousarray(
        bias_int.astype(np.float32).reshape(2, 128).T
    )

    in_maps = []
    for c in range(N_CORES):
        xc = np.ascontiguousarray(
            x_flat[c * IMG_PER_CORE:(c + 1) * IMG_PER_CORE].transpose(1, 2, 0, 3)
        )  # [ci_chunk, 128, img, IMG]
        in_maps.append(
            {
                "x": xc.reshape(2, 128, IMG_PER_CORE * IMG),
                "w": w_t,
                "b": b_t,
            }
        )

    res = run_bass_kernel_spmd(nc, in_maps, core_ids=list(range(N_CORES)))
    LAST_RESULT = res

    y = np.empty((B, C, H, W), dtype=np.int32)
    for c in range(N_CORES):
        yc = res.results[c]["y"]  # [img, co_chunk, 128, H, W]
        for img in range(IMG_PER_CORE):
            y[c * IMG_PER_CORE + img] = yc[img].reshape(C, H, W)
    return y


# revision 9
# speedup vs baseline: 1.0619x; 1.0074x over previous
"""Int32 3x3 conv2d (stride 1, pad 1) as bf16 matmuls on 8 TRN2 cores.

Problem: x[16,256,56,56] (*) w[256,256,3,3] + b[256] -> y[16,256,56,56],
all int32, values in [0,127).

Trick: values 0..126 are exactly representable in bf16, every product is
an integer < 2^14, and every accumulation stays < 2^24, so a bf16 matmul
with fp32 PSUM accumulation produces bit-exact integer results.

Layout: each image is zero-padded to 58x58. The 3x3 conv becomes 9
shifted [Cin,Cout]^T @ [Cin,pixels] matmuls accumulated in PSUM; pixel
tiles are 8 output rows x 56 cols = 448 columns (one PSUM bank), read
from the padded image through a strided access pattern so only valid
pixels are computed.

Sharding: data-parallel over batch, 2 images per core; weights replicated.
"""

import numpy as np
import ml_dtypes

B, C, H, W = 16, 256, 56, 56
HP, WP = H + 2, W + 2          # 58, 58 padded
IMG = HP * WP                  # 3364 flat padded image
N_CORES = 8
IMG_PER_CORE = B // N_CORES    # 2
ROWS_PER_CHUNK = 8
CHUNK = ROWS_PER_CHUNK * W     # 448 valid pixels, fits one PSUM bank
N_CHUNKS = H // ROWS_PER_CHUNK  # 7
N_WARM = 14                    # matmuls to flip the HAM clock gate and
                               # bridge the input-DMA window

_BF16 = ml_dtypes.bfloat16


def _build_program():
    import concourse.bass as bass
    import concourse.mybir as mybir
    from concourse import bacc
    from concourse.tile import TileContext

    nc = bacc.Bacc("TRN2", target_bir_lowering=False, debug=False)

    x_h = nc.dram_tensor(
        "x", [2, 128, IMG_PER_CORE * IMG], mybir.dt.bfloat16,
        kind="ExternalInput",
    )
    w_h = nc.dram_tensor(
        "w", [128, 2 * 2 * 9 * 128], mybir.dt.bfloat16, kind="ExternalInput"
    )
    b_h = nc.dram_tensor("b", [128, 2], mybir.dt.float32, kind="ExternalInput")
    y_h = nc.dram_tensor(
        "y", [IMG_PER_CORE, 2, 128, H, W], mybir.dt.int32, kind="ExternalOutput"
    )

    with TileContext(nc) as tc:
        with (
            tc.tile_pool(name="const", bufs=1) as const_pool,
            tc.tile_pool(name="xin", bufs=1) as x_pool,
            tc.tile_pool(name="psum", bufs=4, space="PSUM") as psum_pool,
            tc.tile_pool(name="warm", bufs=1, space="PSUM") as warm_pool,
            tc.tile_pool(name="outs", bufs=2) as out_pool,
        ):
            # PE warm-up: ~3.4us of junk matmuls on a zeroed tile while the
            # input DMAs land, so the HAM clock gate is at 8/8 (2.4 GHz)
            # when the real matmuls start.
            wz = const_pool.tile([128, 128 + CHUNK], mybir.dt.bfloat16)
            nc.vector.memset(wz[:, :], 0.0)
            wps = warm_pool.tile([128, CHUNK], mybir.dt.float32)
            for i in range(N_WARM):
                nc.tensor.matmul(
                    wps[:, :], wz[:, 0:128], wz[:, 128:128 + CHUNK],
                    start=True, stop=True,
                )

            # Input DMAs: one w tile per (ci_chunk, co_chunk) and one x tile
            # per (ci_chunk, img), so each matmul gates on exactly the data
            # it reads. Issues are spread across engine sequencers (a DMA
            # trigger costs ~0.6us of sequencer time) with the first matmul
            # group's tensors (w00, x00) issued first and in parallel.
            w_sb = {}
            for ci in range(2):
                for co in range(2):
                    w_sb[ci, co] = const_pool.tile(
                        [128, 9 * 128], mybir.dt.bfloat16,
                        tag=f"w_{ci}_{co}", name=f"w_{ci}_{co}",
                    )

            def w_dma(eng, ci, co):
                s = (ci * 2 + co) * 9 * 128
                eng.dma_start(w_sb[ci, co][:, :], w_h.ap()[:, s:s + 9 * 128])

            x_t = {}
            for img in range(IMG_PER_CORE):
                for ci in range(2):
                    x_t[ci, img] = x_pool.tile(
                        [128, IMG], mybir.dt.bfloat16,
                        tag=f"x_{ci}_{img}", name=f"x_{ci}_{img}",
                    )

            def x_dma(eng, ci, img):
                eng.dma_start(
                    x_t[ci, img][:, :],
                    x_h.ap()[ci][:, img * IMG:(img + 1) * IMG],
                )

            b_sb = const_pool.tile([128, 2], mybir.dt.float32)

            # Two issue streams in first-needed order: DMA queues are FIFO,
            # so earlier transfers drain at full bandwidth before later
            # ones start, instead of fair-sharing with not-yet-needed data.
            x_dma(nc.sync, 0, 0)
            w_dma(nc.scalar, 0, 0)
            x_dma(nc.sync, 1, 0)
            w_dma(nc.scalar, 1, 0)
            x_dma(nc.sync, 0, 1)
            w_dma(nc.scalar, 0, 1)
            x_dma(nc.sync, 1, 1)
            w_dma(nc.scalar, 1, 1)
            nc.scalar.dma_start(b_sb[:, :], b_h.ap())

            x_sb = {
                k: t[:, :].rearrange("p (r c) -> p r c", c=WP)
                for k, t in x_t.items()
            }

            for img in range(IMG_PER_CORE):
                for co in range(2):
                    for pc in range(N_CHUNKS):
                        r0 = pc * ROWS_PER_CHUNK
                        ps = psum_pool.tile([128, CHUNK], mybir.dt.float32)
                        n_mm = 0
                        for ci in range(2):
                            for k in range(9):
                                kh, kw = divmod(k, 3)
                                lhsT = w_sb[ci, co][:, k * 128:(k + 1) * 128]
                                rhs = x_sb[ci, img][
                                    :, r0 + kh:r0 + kh + ROWS_PER_CHUNK,
                                    kw:kw + W,
                                ]
                                nc.tensor.matmul(
                                    ps[:, :], lhsT, rhs,
                                    start=(n_mm == 0), stop=(n_mm == 17),
                                )
                                n_mm += 1
                        ot = out_pool.tile([128, CHUNK], mybir.dt.int32)
                        nc.vector.tensor_scalar_add(
                            ot[:, :], ps[:, :], b_sb[:, co:co + 1]
                        )
                        dst = y_h.ap()[img, co].rearrange("p h w -> p (h w)")[
                            :, pc * CHUNK:(pc + 1) * CHUNK
                        ]
                        nc.sync.dma_start(dst, ot[:, :])

    nc.compile()
    return nc


_NC = None
LAST_RESULT = None  # BassKernelResults of the most recent run (for harnesses)


def kernel(x_int: np.ndarray, weight_int: np.ndarray, bias_int: np.ndarray):
    from concourse.bass_utils import run_bass_kernel_spmd

    global _NC, LAST_RESULT
    if _NC is None:
        _NC = _build_program()
    nc = _NC

    x_int = np.asarray(x_int)
    weight_int = np.asarray(weight_int)
    bias_int = np.asarray(bias_int)

    # x: pad to 58x58, cast to bf16, split channels into two 128-partition
    # chunks: per core [ci_chunk, 128, img, IMG].
    x_pad = np.zeros((B, C, HP, WP), dtype=_BF16)
    x_pad[:, :, 1:57, 1:57] = x_int.astype(_BF16)
    x_flat = x_pad.reshape(B, 2, 128, IMG)

    # w[co,ci,kh,kw] -> [ci_part, ci_chunk, co_chunk, k, co_part]
    w_t = (
        weight_int.astype(_BF16)
        .reshape(2, 128, 2, 128, 9)          # [co_c, co_p, ci_c, ci_p, k]
        .transpose(3, 2, 0, 4, 1)            # [ci_p, ci_c, co_c, k, co_p]
        .reshape(128, 2 * 2 * 9 * 128)
    )
    w_t = np.ascontiguousarray(w_t)
    b_t = np.ascontiguousarray(
        bias_int.astype(np.float32).reshape(2, 128).T
    )

    in_maps = []
    for c in range(N_CORES):
        xc = np.ascontiguousarray(
            x_flat[c * IMG_PER_CORE:(c + 1) * IMG_PER_CORE].transpose(1, 2, 0, 3)
        )  # [ci_chunk, 128, img, IMG]
        in_maps.append(
            {
                "x": xc.reshape(2, 128, IMG_PER_CORE * IMG),
                "w": w_t,
                "b": b_t,
            }
        )

    res = run_bass_kernel_spmd(nc, in_maps, core_ids=list(range(N_CORES)))
    LAST_RESULT = res

    y = np.empty((B, C, H, W), dtype=np.int32)
    for c in range(N_CORES):
        yc = res.results[c]["y"]  # [img, co_chunk, 128, H, W]
        for img in range(IMG_PER_CORE):
            y[c * IMG_PER_CORE + img] = yc[img].reshape(C, H, W)
    return y


# revision 11
# speedup vs baseline: 1.0653x; 1.0032x over previous
"""Int32 3x3 conv2d (stride 1, pad 1) as bf16 matmuls on 8 TRN2 cores.

Problem: x[16,256,56,56] (*) w[256,256,3,3] + b[256] -> y[16,256,56,56],
all int32, values in [0,127).

Trick: values 0..126 are exactly representable in bf16, every product is
an integer < 2^14, and every accumulation stays < 2^24, so a bf16 matmul
with fp32 PSUM accumulation produces bit-exact integer results.

Layout: each image is zero-padded to 58x58. The 3x3 conv becomes 9
shifted [Cin,Cout]^T @ [Cin,pixels] matmuls accumulated in PSUM; pixel
tiles are 8 output rows x 56 cols = 448 columns (one PSUM bank), read
from the padded image through a strided access pattern so only valid
pixels are computed.

Sharding: data-parallel over batch, 2 images per core; weights replicated.
"""

import numpy as np
import ml_dtypes

B, C, H, W = 16, 256, 56, 56
HP, WP = H + 2, W + 2          # 58, 58 padded
IMG = HP * WP                  # 3364 flat padded image
N_CORES = 8
IMG_PER_CORE = B // N_CORES    # 2
ROWS_PER_CHUNK = 8
CHUNK = ROWS_PER_CHUNK * W     # 448 valid pixels, fits one PSUM bank
N_CHUNKS = H // ROWS_PER_CHUNK  # 7
N_WARM = 14                    # matmuls to flip the HAM clock gate and
                               # bridge the input-DMA window

_BF16 = ml_dtypes.bfloat16


def _build_program():
    import concourse.bass as bass
    import concourse.mybir as mybir
    from concourse import bacc
    from concourse.tile import TileContext

    nc = bacc.Bacc("TRN2", target_bir_lowering=False, debug=False)

    x_h = nc.dram_tensor(
        "x", [2, 128, IMG_PER_CORE * IMG], mybir.dt.bfloat16,
        kind="ExternalInput",
    )
    w_h = nc.dram_tensor(
        "w", [128, 2 * 2 * 9 * 128], mybir.dt.bfloat16, kind="ExternalInput"
    )
    b_h = nc.dram_tensor("b", [128, 2], mybir.dt.float32, kind="ExternalInput")
    y_h = nc.dram_tensor(
        "y", [IMG_PER_CORE, 2, 128, H, W], mybir.dt.int32, kind="ExternalOutput"
    )

    with TileContext(nc) as tc:
        with (
            tc.tile_pool(name="const", bufs=1) as const_pool,
            tc.tile_pool(name="xin", bufs=1) as x_pool,
            tc.tile_pool(name="psum", bufs=5, space="PSUM") as psum_pool,
            tc.tile_pool(name="warm", bufs=1, space="PSUM") as warm_pool,
            tc.tile_pool(name="outs", bufs=2) as out_pool,
        ):
            # PE warm-up: ~3.4us of junk matmuls on a zeroed tile while the
            # input DMAs land, so the HAM clock gate is at 8/8 (2.4 GHz)
            # when the real matmuls start.
            wz = const_pool.tile([128, 128 + CHUNK], mybir.dt.bfloat16)
            nc.vector.memset(wz[:, :], 0.0)
            wps = warm_pool.tile([128, CHUNK], mybir.dt.float32)
            for i in range(N_WARM):
                nc.tensor.matmul(
                    wps[:, :], wz[:, 0:128], wz[:, 128:128 + CHUNK],
                    start=True, stop=True,
                )

            # Input DMAs: one w tile per (ci_chunk, co_chunk) and one x tile
            # per (ci_chunk, img), so each matmul gates on exactly the data
            # it reads. Issues are spread across engine sequencers (a DMA
            # trigger costs ~0.6us of sequencer time) with the first matmul
            # group's tensors (w00, x00) issued first and in parallel.
            w_sb = {}
            for ci in range(2):
                for co in range(2):
                    w_sb[ci, co] = const_pool.tile(
                        [128, 9 * 128], mybir.dt.bfloat16,
                        tag=f"w_{ci}_{co}", name=f"w_{ci}_{co}",
                    )

            def w_dma(eng, ci, co):
                s = (ci * 2 + co) * 9 * 128
                eng.dma_start(w_sb[ci, co][:, :], w_h.ap()[:, s:s + 9 * 128])

            x_t = {}
            for img in range(IMG_PER_CORE):
                for ci in range(2):
                    x_t[ci, img] = x_pool.tile(
                        [128, IMG], mybir.dt.bfloat16,
                        tag=f"x_{ci}_{img}", name=f"x_{ci}_{img}",
                    )

            def x_dma(eng, ci, img):
                eng.dma_start(
                    x_t[ci, img][:, :],
                    x_h.ap()[ci][:, img * IMG:(img + 1) * IMG],
                )

            b_sb = const_pool.tile([128, 2], mybir.dt.float32)

            # Two issue streams in first-needed order: DMA queues are FIFO,
            # so earlier transfers drain at full bandwidth before later
            # ones start, instead of fair-sharing with not-yet-needed data.
            x_dma(nc.sync, 0, 0)
            w_dma(nc.scalar, 0, 0)
            x_dma(nc.sync, 1, 0)
            w_dma(nc.scalar, 1, 0)
            x_dma(nc.sync, 0, 1)
            w_dma(nc.scalar, 0, 1)
            x_dma(nc.sync, 1, 1)
            w_dma(nc.scalar, 1, 1)
            nc.scalar.dma_start(b_sb[:, :], b_h.ap())

            x_sb = {
                k: t[:, :].rearrange("p (r c) -> p r c", c=WP)
                for k, t in x_t.items()
            }

            def mm(ps, ci, co, img, r0, rows, start, stop):
                for k in range(9):
                    kh, kw = divmod(k, 3)
                    nc.tensor.matmul(
                        ps[:, :],
                        w_sb[ci, co][:, k * 128:(k + 1) * 128],
                        x_sb[ci, img][:, r0 + kh:r0 + kh + rows, kw:kw + W],
                        start=start and k == 0,
                        stop=stop and k == 8,
                    )

            def epilogue(ps, co, img, r0, rows):
                n = rows * W
                ot = out_pool.tile([128, CHUNK], mybir.dt.int32, tag="ot")
                nc.vector.tensor_scalar_add(
                    ot[:, :n], ps[:, :], b_sb[:, co:co + 1]
                )
                dst = y_h.ap()[img, co].rearrange("p h w -> p (h w)")[
                    :, r0 * W:r0 * W + n
                ]
                nc.sync.dma_start(dst, ot[:, :n])

            # First plane: sweep ci=0 over the first 4 chunks before any
            # ci=1 matmul, so the PE only gates on the first x and w
            # transfers (w00+x00) instead of all four.
            HEAD = 4
            head_ps = []
            for pc in range(HEAD):
                ps = psum_pool.tile([128, CHUNK], mybir.dt.float32, tag="ps",
                                    name=f"ps_h{pc}")
                head_ps.append(ps)
                mm(ps, 0, 0, 0, pc * ROWS_PER_CHUNK, ROWS_PER_CHUNK,
                   start=True, stop=False)
            for pc in range(HEAD):
                mm(head_ps[pc], 1, 0, 0, pc * ROWS_PER_CHUNK, ROWS_PER_CHUNK,
                   start=False, stop=True)
                epilogue(head_ps[pc], 0, 0, pc * ROWS_PER_CHUNK,
                         ROWS_PER_CHUNK)

            # chunk row-splits per (img, co) plane; the globally last chunk
            # is split [6, 2] so the final PSUM->SBUF->HBM drain is short
            for img in range(IMG_PER_CORE):
                for co in range(2):
                    if img == 0 and co == 0:
                        chunks = [(pc * ROWS_PER_CHUNK, ROWS_PER_CHUNK)
                                  for pc in range(HEAD, N_CHUNKS)]
                    elif img == IMG_PER_CORE - 1 and co == 1:
                        chunks = [(pc * ROWS_PER_CHUNK, ROWS_PER_CHUNK)
                                  for pc in range(N_CHUNKS - 1)]
                        chunks += [(48, 6), (54, 2)]
                    else:
                        chunks = [(pc * ROWS_PER_CHUNK, ROWS_PER_CHUNK)
                                  for pc in range(N_CHUNKS)]
                    for r0, rows in chunks:
                        ps = psum_pool.tile([128, CHUNK], mybir.dt.float32,
                                            tag="ps", name=f"ps_{img}_{co}_{r0}")
                        mm(ps[:, :rows * W], 0, co, img, r0, rows,
                           start=True, stop=False)
                        mm(ps[:, :rows * W], 1, co, img, r0, rows,
                           start=False, stop=True)
                        epilogue(ps[:, :rows * W], co, img, r0, rows)

    nc.compile()
    return nc


_NC = None
LAST_RESULT = None  # BassKernelResults of the most recent run (for harnesses)


def kernel(x_int: np.ndarray, weight_int: np.ndarray, bias_int: np.ndarray):
    from concourse.bass_utils import run_bass_kernel_spmd

    global _NC, LAST_RESULT
    if _NC is None:
        _NC = _build_program()
    nc = _NC

    x_int = np.asarray(x_int)
    weight_int = np.asarray(weight_int)
    bias_int = np.asarray(bias_int)

    # x: pad to 58x58, cast to bf16, split channels into two 128-partition
    # chunks: per core [ci_chunk, 128, img, IMG].
    x_pad = np.zeros((B, C, HP, WP), dtype=_BF16)
    x_pad[:, :, 1:57, 1:57] = x_int.astype(_BF16)
    x_flat = x_pad.reshape(B, 2, 128, IMG)

    # w[co,ci,kh,kw] -> [ci_part, ci_chunk, co_chunk, k, co_part]
    w_t = (
        weight_int.astype(_BF16)
        .reshape(2, 128, 2, 128, 9)          # [co_c, co_p, ci_c, ci_p, k]
        .transpose(3, 2, 0, 4, 1)            # [ci_p, ci_c, co_c, k, co_p]
        .reshape(128, 2 * 2 * 9 * 128)
    )
    w_t = np.ascontiguousarray(w_t)
    b_t = np.ascontiguousarray(
        bias_int.astype(np.float32).reshape(2, 128).T
    )

    in_maps = []
    for c in range(N_CORES):
        xc = np.ascontiguousarray(
            x_flat[c * IMG_PER_CORE:(c + 1) * IMG_PER_CORE].transpose(1, 2, 0, 3)
        )  # [ci_chunk, 128, img, IMG]
        in_maps.append(
            {
                "x": xc.reshape(2, 128, IMG_PER_CORE * IMG),
                "w": w_t,
                "b": b_t,
            }
        )

    res = run_bass_kernel_spmd(nc, in_maps, core_ids=list(range(N_CORES)))
    LAST_RESULT = res

    y = np.empty((B, C, H, W), dtype=np.int32)
    for c in range(N_CORES):
        yc = res.results[c]["y"]  # [img, co_chunk, 128, H, W]
        for img in range(IMG_PER_CORE):
            y[c * IMG_PER_CORE + img] = yc[img].reshape(C, H, W)
    return y
